# revision 22
# baseline (speedup 1.0000x reference)
"""J-regularized cross-entropy loss on 8 Trainium2 cores.

Per core (2 batches, N=262144 px, C=8): host sorts each batch's pixels
by target class and pads every class run to RC cols (32 px/col, zero
pixels), so the device program is data-independent. Layout: two SBUF
tiles (half h = classes 4h..4h+3), partition = 32*(c%4) + slot,
free = pixel column. All pred ships as fp8e3 (4.3 MB/core, one stream).

Device:
  codes[p, t] = int16(round(A*x + B))       (Schraudolph exp bits)
      DVE tensor_scalar fp8->int16 (2x_2P mode, ~0.54 ns/el) for ~73%
      of columns, ACT Copy scale/bias (1 el/cyc) striped over the rest
      so both engines drain each DMA arrival together.
  lse:  blockones [128,32] matmul pairs (halves accumulate) ->
      psum [32,512] regions, 4 col-tiled regions/bank (concurrent via
      tile_position); ACT Ln over [128,1024] psum with accum_out ->
      per-partition lse sums.
  S[b,k,c] = sum of class-k pred: ones4 [128,4] matmul over the raw
      fp8 pred tiles, output [4, L/2, 2] stride-(0,1) psum cells
      (free-dim accumulation via has_written; 8-byte psum cachelines
      coalesce, hence the column pairs), one cell pair per (run, half);
      zero-weight dummy MM pre-clears the bank, units use start=False.

Host: unpack S cells, lse total minus the (fixed 16384/core) zero-pad
pixels' ln(8*v0), then M = S^T/n, jl, ce exactly as the reference.
Baseline 46.9us -> this version ~37.5us (DMA 4.33MB @ ~330GB/s plus
~7us fixed preamble and ~4us postamble dominate; DVE/ACT/PE each run
~15us inside the DMA window).
"""

import numpy as np
import ml_dtypes

import concourse.bacc as bacc
import concourse.mybir as mybir
import concourse.tile as tile
from concourse import bass_utils

N_CORES = 8
B, C, H, W = 16, 8, 512, 512
N = H * W
P = 128
BPC = B // N_CORES        # batches per core
SLOTS = 32                # pixels per column
CHUNK = 512               # cols per blockones matmul
NRUNS = BPC * C           # class runs per core (16)

LOG2E = 1.4426950408889634
SCHRAU_A = 128.0 * LOG2E
SCHRAU_B = 16256.0 - 7.368
V0 = 0.97265625           # int16 16249 viewed as bf16 (code of x=0)
ACT_SET_NL = 6            # natural_log_exp_and_others

def _piece_sizes(T):
    """Tapered DMA piece sizes per half: small head for an early compute
    start; few pieces overall (each dma_start costs ~680ns of serial
    descriptor generation on the Sync engine)."""
    head = [512, 1536, 2560]
    mid_total = T - sum(head)
    m = (mid_total // 3) & ~3
    return head + [m, m, mid_total - 2 * m]


def _code_calls(T):
    """(engine, lo, hi) code-generation calls per half, in column order.
    The stream is split into windows; ACT takes ~27% at the end of each
    window so both engines drain each arrival (and the tail) together."""
    def r4(x):
        return int(x) & ~3
    w = [0, r4(0.27 * T), r4(0.52 * T), r4(0.76 * T), T]
    calls = []
    for i in range(4):
        lo, hi = w[i], w[i + 1]
        if i == 3:
            # last window: ACT first so it is free for the Ln chain,
            # DVE drains the very tail
            split = r4(lo + 0.27 * (hi - lo))
            calls.append(("act", lo, split))
            mid = r4((split + hi) / 2)
            calls.append(("dve", split, mid))
            calls.append(("dve", mid, hi))
            break
        split = r4(hi - 0.27 * (hi - lo))
        if i == 0:
            calls.append(("dve", 0, 512))
            calls.append(("dve", 512, 2048))
            calls.append(("dve", 2048, split))
        else:
            mid = r4((lo + split) / 2)
            calls.append(("dve", lo, mid))
            calls.append(("dve", mid, split))
        calls.append(("act", split, hi))
    return calls

TRACE = False
LAST_EXEC_NS = None
LAST_TRACE = None

_BF16 = mybir.dt.bfloat16
_F32 = mybir.dt.float32
_F8E3 = mybir.dt.float8e3
_I16 = mybir.dt.int16

_nc_cache = {}


def _build_nc(RC):
    T = NRUNS * RC                      # cols per core (multiple of 512)
    NCHK = T // CHUNK
    sizes = _piece_sizes(T)
    offs = np.concatenate([[0], np.cumsum(sizes)]).astype(int)
    NGRP = (NCHK + 7) // 8              # ln groups ([128,1024] psum tiles)
    extra_ln = 1 if (NCHK % 8) in (5, 6, 7) else 0
    NOUT = 16 + NGRP + extra_ln

    nc = bacc.Bacc("TRN2", target_bir_lowering=False, debug=False,
                   num_devices=N_CORES)
    pk_d = nc.dram_tensor("pk", (P, 2 * T), mybir.dt.uint8,
                          kind="ExternalInput")
    w_d = nc.dram_tensor("w", (P, 64), _BF16, kind="ExternalInput")
    out_d = nc.dram_tensor("out", (P, NOUT), _F32, kind="ExternalOutput")

    with tile.TileContext(nc) as tc:
        nc.scalar.add_instruction(mybir.InstLoadActFuncSet(
            name=nc.get_next_instruction_name(),
            act_func_set_id=ACT_SET_NL, ins=[], outs=[]))
        with (
            tc.tile_pool(name="big", bufs=1) as big,
            tc.tile_pool(name="lse", bufs=2, space="PSUM") as lse_pool,
            tc.tile_pool(name="s4", bufs=1, space="PSUM") as s4_pool,
        ):
            pk = big.tile([P, 2 * T], mybir.dt.uint8, name="pk")
            codes = big.tile([P, 2 * T], _I16, name="codes")
            w = big.tile([P, 64], _BF16, name="w")
            out_sb = big.tile([P, NOUT], _F32, name="out_sb")
            lnt = big.tile([P, 1024], _BF16, name="lnt")

            bo = w[:, 0:32]     # blockones: w[p, p%32] = 1
            o4 = w[:, 32:36]    # ones4: w[p, 32 + p//32] = 1
            zo = w[:, 36:37]    # zeros

            def p8(h, c0, c1):
                return pk[:, h * T + c0: h * T + c1].bitcast(_F8E3)

            def cb(h, c0, c1):
                return codes[:, h * T + c0: h * T + c1].bitcast(_BF16)

            nc.sync.dma_start(w[:, :], w_d[:, :])

            # input DMA, interleaving halves so early cols of both
            # halves land first
            for pc in range(len(sizes)):
                for h in range(2):
                    lo = h * T + offs[pc]
                    hi = h * T + offs[pc + 1]
                    nc.sync.dma_start(pk[:, lo:hi], pk_d[:, lo:hi])

            # S accumulator bank: pre-clear via zero matmul
            s4 = s4_pool.tile([P, 16], _F32, name="s4")
            nc.tensor.matmul(s4[:, :], zo.broadcast_to([P, 128]),
                             w[:, 0:16], start=True, stop=False,
                             skip_group_check=True)

            # codes generation (call slicing decoupled from DMA pieces)
            for eng, c0, c1 in _code_calls(T):
                for h in range(2):
                    lo, hi = h * T + c0, h * T + c1
                    src = pk[:, lo:hi].bitcast(_F8E3)
                    dst = codes[:, lo:hi]
                    if eng == "act":
                        nc.scalar.activation(
                            dst, src, mybir.ActivationFunctionType.Copy,
                            bias=SCHRAU_B, scale=SCHRAU_A)
                    else:
                        nc.vector.tensor_scalar(
                            dst, src, SCHRAU_A, SCHRAU_B,
                            mybir.AluOpType.mult, mybir.AluOpType.add)

            # ones4 matmul slices per (run, half) unit
            def ones4_unit(r, h):
                u = r * 2 + h
                q = u % 4
                cp = u // 4
                cell = s4[32 * q:32 * q + 4, 2 * cp:2 * cp + 2]
                c0 = r * RC
                slices = []
                while c0 < (r + 1) * RC:
                    ln_ = min(512, (r + 1) * RC - c0)
                    slices.append((c0, c0 + ln_))
                    c0 += ln_
                for si, (a, b_) in enumerate(slices):
                    outap = cell.unsqueeze(1).broadcast_to(
                        [4, (b_ - a) // 2, 2])
                    nc.tensor.matmul(
                        outap, o4, p8(h, a, b_),
                        start=False, stop=(si == len(slices) - 1),
                        tile_position=(0, 32 * q),
                        skip_group_check=True)

            # blockones matmuls + ln per psum group, with each run's
            # ones4 units (which need only raw pred, no codes) emitted
            # as soon as the chunk covering the run's end is reached
            for g in range(NGRP):
                ps = lse_pool.tile([P, 1024], _F32, tag="lse", name="ps")
                nch = min(8, NCHK - g * 8)
                for jj in range(nch):
                    j = g * 8 + jj
                    for r in range(NRUNS):
                        if j * CHUNK < (r + 1) * RC <= (j + 1) * CHUNK:
                            ones4_unit(r, 0)
                            ones4_unit(r, 1)
                    r = jj % 4
                    colh = jj // 4
                    reg = ps[32 * r:32 * r + 32,
                             512 * colh:512 * colh + 512]
                    a = j * CHUNK
                    nc.tensor.matmul(reg, bo, cb(0, a, a + CHUNK),
                                     start=True, stop=False,
                                     tile_position=(0, 32 * r))
                    nc.tensor.matmul(reg, bo, cb(1, a, a + CHUNK),
                                     start=False, stop=True,
                                     tile_position=(0, 32 * r))
                if nch == 8:
                    nc.scalar.activation(
                        lnt[:, 0:1024], ps[:, 0:1024],
                        mybir.ActivationFunctionType.Ln,
                        accum_out=out_sb[:, 16 + g:17 + g])
                elif nch <= 4:
                    nc.scalar.activation(
                        lnt[0:32 * nch, 0:512], ps[0:32 * nch, 0:512],
                        mybir.ActivationFunctionType.Ln,
                        accum_out=out_sb[0:32 * nch, 16 + g:17 + g])
                else:
                    nc.scalar.activation(
                        lnt[:, 0:512], ps[:, 0:512],
                        mybir.ActivationFunctionType.Ln,
                        accum_out=out_sb[:, 16 + g:17 + g])
                    nc.scalar.activation(
                        lnt[0:32 * (nch - 4), 512:1024],
                        ps[0:32 * (nch - 4), 512:1024],
                        mybir.ActivationFunctionType.Ln,
                        accum_out=out_sb[0:32 * (nch - 4),
                                         16 + NGRP:17 + NGRP])

            nc.vector.tensor_copy(out_sb[:, 0:16], s4[:, :])
            nc.sync.dma_start(out_d[:, :], out_sb[:, :],
                              single_packet=True)

    nc.compile()
    return nc


def kernel(pred, target):
    global LAST_EXEC_NS, LAST_TRACE
    pred = np.asarray(pred)
    target = np.asarray(target)

    pred8 = pred.astype(ml_dtypes.float8_e3m4)   # quantize once, full

    # per-batch class counts and sorted orders
    orders, counts = [], []
    for b in range(B):
        t = target[b].reshape(N).astype(np.int64)
        orders.append(np.argsort(t, kind="stable"))
        counts.append(np.bincount(t, minlength=C))
    counts = np.array(counts)                     # (B, C)

    RC = 32 * int(np.ceil(counts.max() / 1024.0))
    T = NRUNS * RC

    if RC not in _nc_cache:
        _nc_cache[RC] = _build_nc(RC)
    nc = _nc_cache[RC]

    wmat = np.zeros((P, 64), dtype=np.float32)
    for p in range(P):
        wmat[p, p % 32] = 1.0
        wmat[p, 32 + p // 32] = 1.0
    wmat = wmat.astype(ml_dtypes.bfloat16)

    in_maps = []
    for core in range(N_CORES):
        cols = []
        for bb in range(BPC):
            b = core * BPC + bb
            pb8 = pred8[b].reshape(C, N)
            z = np.zeros((C, 1), dtype=pb8.dtype)
            pbx = np.concatenate([pb8, z], axis=1)    # sentinel col
            idx = np.full((C, RC * 32), N, dtype=np.int64)
            ofs = 0
            for k in range(C):
                nk = counts[b, k]
                idx[k, :nk] = orders[b][ofs:ofs + nk]
                ofs += nk
            cols.append(pbx[:, idx.reshape(-1)])      # (C, 8*RC*32)
        full = np.concatenate(cols, axis=1)           # (C, T*32)
        arr = full.reshape(C, T, SLOTS)
        halves = []
        for h in range(2):
            a = arr[4 * h:4 * h + 4]                  # (4, T, 32)
            tilearr = a.transpose(0, 2, 1).reshape(P, T)
            halves.append(tilearr.view(np.uint8))
        pkarr = np.ascontiguousarray(
            np.concatenate(halves, axis=1))           # (128, 2T)
        in_maps.append({"pk": pkarr, "w": wmat})

    res = bass_utils.run_bass_kernel_spmd(
        nc, in_maps, core_ids=list(range(N_CORES)), trace=TRACE)
    LAST_EXEC_NS = res.exec_time_ns
    LAST_TRACE = (res.instructions_and_trace[1]
                  if res.instructions_and_trace else None)

    NCHK = T // CHUNK
    NGRP = (NCHK + 7) // 8

    S = np.zeros((B, C, C), np.float64)               # S[b, k, c]
    lse_dev = 0.0
    for core in range(N_CORES):
        out = res.results[core]["out"].astype(np.float64)
        for r in range(NRUNS):
            bb, k = divmod(r, C)
            b = core * BPC + bb
            for h in range(2):
                u = r * 2 + h
                q = u % 4
                cp = u // 4
                cell = out[32 * q:32 * q + 4, 2 * cp:2 * cp + 2]
                S[b, k, 4 * h:4 * h + 4] += cell.sum(axis=1)
        for g in range(NGRP):
            nch = min(8, NCHK - g * 8)
            if nch == 8:
                lse_dev += out[:, 16 + g].sum()
            elif nch <= 4:
                lse_dev += out[0:32 * nch, 16 + g].sum()
            else:
                lse_dev += out[:, 16 + g].sum()
                lse_dev += out[0:32 * (nch - 4), 16 + NGRP].sum()

    n_pads = N_CORES * (T * SLOTS - BPC * N)
    lse_real = lse_dev - n_pads * np.log(C * V0)

    global DBG
    DBG = {"S": S.copy(), "lse_dev": lse_dev, "lse_real": lse_real,
           "counts": counts.copy()}

    n = counts.astype(np.float64)
    M = S.transpose(0, 2, 1) / n[:, None, :]          # M[b, c, k]
    diag = np.einsum("bcc->bc", M)
    inner = (diag[:, :, None] - M) * 0.5
    off = 1.0 - np.eye(C)
    jl = (-(np.log(0.5 + inner) * off).sum(axis=(1, 2))).mean()
    trS = np.einsum("bkk->", S)
    ce = (lse_real - trS) / (B * N)
    return np.float32(jl + ce)
